# revision 3
# baseline (speedup 1.0000x reference)
"""HardGateMOE Trainium2 kernel: expert-parallel across 8 NeuronCores.

Strategy: each core owns one expert (W1[e], W2[e]). The host performs the
"all-to-all token dispatch by mapping": for each expert it gathers the unique
tokens routed to it (padded to a common capacity C), transposed so the token
dim sits on the matmul free axis on device. Each core runs
  hT = gelu(W1[e].T @ xgT + b1)   # [F, C], tokens on free axis
  yT = W2[e].T @ hT + b2          # [H, C], returned as bf16
Expert GEMMs run in bf16 (half the weight-DMA bytes of fp32; fastest legal
PE streaming rate -- fp8 DoubleRow is 2x/matmul but the hi/lo split needed
for the 2e-2 tolerance costs 3 matmuls = net 1.5x slower). The tiny gate
GEMM, token-axis softmax, per-(token,k) gate weights, and weighted
scatter-add combine run on host, keeping the device NEFF a pure two-GEMM
pipeline.

Schedule notes (driven by the TimelineSim cost model):
- One shared 8-buf PSUM pool spans warmup+fc1+fc2 so fc2's accumulators
  only wait for their own bank's last reader, not a pool-close barrier.
- Startup DMAs are ordered xg_k0, w1g0_k0, xg_k12, w1g0_k12, ... so the
  first accumulation chain's operands land at the earliest possible time
  given the serial HWDGE (625ns/DMA) + DGE delay (650) + completion-sem
  (900) pipeline. Biases ride the Pool/SWDGE path which does not contend
  for HWDGE.
- PE warmup is a chain of matmuls on a memset tile sized to end right at
  first-data-ready: the PE p-state ramp (half clock for the first 3us of
  busy) burns on dummy work during the dead DMA window, and the handoff
  to real matmuls is gap-free.
- fc1 groups are emitted k-major (all 4 f-chains advance together) to
  match the k-slice DMA arrival order.
"""

import ml_dtypes
import numpy as np

import concourse.tile as tile
from concourse import bacc, mybir
from concourse.bass_utils import run_bass_kernel_spmd

N, H, F, E = 2048, 1024, 4096, 8
NCORES = 8
P = 128
KH = H // P               # 8  k-chunks for fc1 (contract over H)
KF = F // P               # 32 k-chunks for fc2 (contract over F)
FT = F // P               # 32 f-tiles of hT
HT = H // P               # 8  h-tiles of yT
FG = 8                    # fc1 f-tile groups (4 f-tiles = 512 cols each)
GW = F // FG              # 512 cols of W1 per group
GF = GW // P              # 4 f-tiles per group

TAILF = 6                 # fc2 tail chunks run per-h so output acts/DMAs
                          # stagger instead of piling up at the end
NWARM = 5                 # warmup matmuls (PE p-state ramp burn)

BF16 = ml_dtypes.bfloat16

_compiled = {}
_last_nc = None


def _build(C: int, repeats: int = 1):
    """Build + compile the SPMD program for token capacity C (<=512)."""
    nc = bacc.Bacc("TRN2", target_bir_lowering=False, debug=False,
                   num_devices=NCORES)
    f32 = mybir.dt.float32
    bf16 = mybir.dt.bfloat16
    GELU = mybir.ActivationFunctionType.Gelu_apprx_tanh
    IDENT = mybir.ActivationFunctionType.Identity

    xg = nc.dram_tensor("xg", [P, KH, C], bf16, kind="ExternalInput").ap()
    w1 = nc.dram_tensor("w1", [FG, P, KH, GW], bf16, kind="ExternalInput").ap()
    b1 = nc.dram_tensor("b1", [P, FT], f32, kind="ExternalInput").ap()
    w2 = nc.dram_tensor("w2", [KF, P, H], bf16, kind="ExternalInput").ap()
    b2 = nc.dram_tensor("b2", [P, HT], f32, kind="ExternalInput").ap()
    y = nc.dram_tensor("y", [HT, P, C], bf16, kind="ExternalOutput").ap()

    with tile.TileContext(nc) as tc:
      for _rep in range(repeats):
        with (
            tc.tile_pool(name="singles", bufs=1) as singles,
            tc.tile_pool(name="w1p", bufs=3) as w1p,
            tc.tile_pool(name="w2p", bufs=10) as w2p,
            tc.tile_pool(name="outp", bufs=8) as outp,
            tc.tile_pool(name="ps", bufs=8, space="PSUM") as ps,
        ):
            # PE warmup: a chain of dummy matmuls on a memset tile fills the
            # startup DMA window, burning the p-state ramp (half clock for
            # the first ~3us of PE-busy) on dummy work so real matmuls run
            # at full clock. Sized to end right at first-data-ready so the
            # warmup->real handoff has no PE idle gap.
            warm = singles.tile([P, C], bf16)
            nc.vector.memset(warm, 0.0)
            for i in range(NWARM):
                wps = ps.tile([P, C], f32, tag="ps", name=f"wps{i}")
                nc.tensor.matmul(out=wps, lhsT=warm[:, 0:P], rhs=warm,
                                 start=True, stop=True)

            # Startup feed, all on the SP queue (serial HWDGE): alternate
            # xg/w1 pieces so the k-slices both chains need arrive in
            # consumption order.
            xg_s = singles.tile([P, KH, C], bf16)
            w1_g0 = w1p.tile([P, KH, GW], bf16, name="w1g0", tag="w1")
            nc.sync.dma_start(out=xg_s[:, 0:1, :], in_=xg[:, 0:1, :])
            nc.sync.dma_start(out=w1_g0[:, 0:1, :], in_=w1[0, :, 0:1, :])
            nc.sync.dma_start(out=xg_s[:, 1:3, :], in_=xg[:, 1:3, :])
            nc.sync.dma_start(out=w1_g0[:, 1:3, :], in_=w1[0, :, 1:3, :])
            nc.sync.dma_start(out=xg_s[:, 3:5, :], in_=xg[:, 3:5, :])
            nc.sync.dma_start(out=w1_g0[:, 3:5, :], in_=w1[0, :, 3:5, :])
            nc.sync.dma_start(out=xg_s[:, 5:KH, :], in_=xg[:, 5:KH, :])
            nc.sync.dma_start(out=w1_g0[:, 5:KH, :], in_=w1[0, :, 5:KH, :])
            # Biases ride the Pool/SWDGE path: no HWDGE contention.
            b1_s = singles.tile([P, FT], f32)
            b2_s = singles.tile([P, HT], f32)
            nc.gpsimd.dma_start(out=b1_s, in_=b1)
            nc.gpsimd.dma_start(out=b2_s, in_=b2)
            hT_s = singles.tile([P, FT, C], bf16)

            def fc1_group(fg, w1_t):
                if w1_t is None:
                    w1_t = w1p.tile([P, KH, GW], bf16, name=f"w1g{fg}",
                                    tag="w1")
                    (nc.sync if fg % 2 else nc.scalar).dma_start(
                        out=w1_t[:, 0:4, :], in_=w1[fg, :, 0:4, :])
                    (nc.scalar if fg % 2 else nc.sync).dma_start(
                        out=w1_t[:, 4:KH, :], in_=w1[fg, :, 4:KH, :])
                # k-major emission: all 4 f-chains advance together so
                # consumption matches the k-slice DMA arrival order.
                pst = [ps.tile([P, C], f32, tag="ps", name=f"ps{fg}_{fl}")
                       for fl in range(GF)]
                for k in range(KH):
                    for fl in range(GF):
                        nc.tensor.matmul(
                            out=pst[fl],
                            lhsT=w1_t[:, k, fl * P:(fl + 1) * P],
                            rhs=xg_s[:, k, :],
                            start=(k == 0), stop=(k == KH - 1))
                for fl in range(GF):
                    ft = fg * GF + fl
                    nc.scalar.activation(
                        out=hT_s[:, ft, :], in_=pst[fl], func=GELU,
                        bias=b1_s[:, ft:ft + 1])

            fc1_group(0, w1_g0)
            for fg in range(1, FG):
                fc1_group(fg, None)

            # fc2: 8 accumulators (one per h-tile) over 32 f-chunks. The
            # shared PSUM pool means each ps_y waits only for its own
            # bank's last gelu, so the first fc2 matmul issues right after
            # fc1's last -- no pool-close barrier.
            ps_y = [ps.tile([P, C], f32, tag="ps", name=f"ps_y{h}")
                    for h in range(HT)]
            w2_t = []
            for f in range(KF):
                t = w2p.tile([P, H], bf16, tag="w2", name="w2t")
                (nc.sync if f % 2 == 0 else nc.scalar).dma_start(
                    out=t, in_=w2[f])
                w2_t.append(t)
                if f < KF - TAILF:
                    for h in range(HT):
                        nc.tensor.matmul(
                            out=ps_y[h], lhsT=t[:, h * P:(h + 1) * P],
                            rhs=hT_s[:, f, :], start=(f == 0), stop=False)
            for h in range(HT):
                for f in range(KF - TAILF, KF):
                    nc.tensor.matmul(
                        out=ps_y[h], lhsT=w2_t[f][:, h * P:(h + 1) * P],
                        rhs=hT_s[:, f, :], start=False, stop=(f == KF - 1))
                o_t = outp.tile([P, C], bf16, tag="y", name=f"o{h}")
                nc.scalar.activation(out=o_t, in_=ps_y[h], func=IDENT,
                                     bias=b2_s[:, h:h + 1])
                (nc.sync if h % 2 == 0 else nc.scalar).dma_start(
                    out=y[h], in_=o_t)

    nc.compile()
    return nc


def kernel(**inputs) -> np.ndarray:
    global _last_nc
    x = np.ascontiguousarray(np.asarray(inputs["x"], dtype=np.float32))
    mapping = np.asarray(inputs["mapping"]).astype(np.int64)
    Wg = np.asarray(inputs["Wg"], dtype=np.float32)
    W1 = np.asarray(inputs["W1"], dtype=np.float32)
    b1 = np.asarray(inputs["b1"], dtype=np.float32)
    W2 = np.asarray(inputs["W2"], dtype=np.float32)
    b2 = np.asarray(inputs["b2"], dtype=np.float32)

    n, h = x.shape
    assert (n, h) == (N, H)

    # Host-side dispatch: unique tokens per expert (a token routed to the
    # same expert by both slots contributes once, with summed gate weight).
    token_lists = []
    for e in range(E):
        tl = np.nonzero((mapping == e).any(axis=1))[0]
        token_lists.append(tl)
    maxc = max(len(tl) for tl in token_lists)
    C = max(256, -(-maxc // 8) * 8)
    assert C <= 512, f"per-expert token count {maxc} exceeds single-chunk capacity"

    if C not in _compiled:
        _compiled[C] = _build(C)
    nc = _compiled[C]
    _last_nc = nc

    in_maps = []
    for e in range(E):
        tl = token_lists[e]
        xgT = np.zeros((H, C), dtype=BF16)
        xgT[:, :len(tl)] = x[tl].T.astype(BF16)
        in_maps.append({
            # [P, KH, C]: xg[r, k, c] = x[tl[c], k*128+r]
            "xg": np.ascontiguousarray(xgT.reshape(KH, P, C).transpose(1, 0, 2)),
            # [FG, P, KH, GW]: w1[fg, r, k, c] = W1[k*128+r, fg*512+c]
            "w1": np.ascontiguousarray(
                W1[e].reshape(KH, P, FG, GW).transpose(2, 1, 0, 3)).astype(BF16),
            "b1": np.ascontiguousarray(b1[e].reshape(FT, P).T),
            "w2": W2[e].reshape(KF, P, H).astype(BF16),
            "b2": np.ascontiguousarray(b2[e].reshape(HT, P).T),
        })

    res = run_bass_kernel_spmd(nc, in_maps, list(range(NCORES)))

    # Host: gate GEMM + token-axis softmax, per-(token,k) weights, combine.
    zf = x @ Wg.T                                     # [N, E]
    zf -= zf.max(axis=0, keepdims=True)
    ez = np.exp(zf)
    logits = ez / ez.sum(axis=0, keepdims=True)
    w = np.take_along_axis(logits, mapping, axis=1)
    w = w / w.sum(axis=1, keepdims=True)

    out = np.zeros((N, H), dtype=np.float32)
    for e in range(E):
        tl = token_lists[e]
        yT = res.results[e]["y"].astype(np.float32).reshape(H, -1)
        cw = (w[tl, 0] * (mapping[tl, 0] == e)
              + w[tl, 1] * (mapping[tl, 1] == e)).astype(np.float32)
        out[tl] += cw[:, None] * yT[:, :len(tl)].T
    return out


# revision 5
# speedup vs baseline: 1.0303x; 1.0303x over previous
"""HardGateMOE Trainium2 kernel: expert-parallel across 8 NeuronCores.

Strategy: each core owns one expert (W1[e], W2[e]). The host performs the
"all-to-all token dispatch by mapping": for each expert it gathers the unique
tokens routed to it (padded to a common capacity C), transposed so the token
dim sits on the matmul free axis on device. Each core runs
  hT = gelu(W1[e].T @ xgT + b1)   # [F, C], tokens on free axis
  yT = W2[e].T @ hT + b2          # [H, C], returned as bf16
Expert GEMMs run in bf16 (half the weight-DMA bytes of fp32; fastest legal
PE streaming rate -- fp8 DoubleRow is 2x/matmul but the hi/lo split needed
for the 2e-2 tolerance costs 3 matmuls = net 1.5x slower). The tiny gate
GEMM, token-axis softmax, per-(token,k) gate weights, and weighted
scatter-add combine run on host, keeping the device NEFF a pure two-GEMM
pipeline.

Schedule notes (driven by the TimelineSim cost model):
- One shared 8-buf PSUM pool spans warmup+fc1+fc2 so fc2's accumulators
  only wait for their own bank's last reader, not a pool-close barrier.
- Startup DMAs are ordered xg_k0, w1g0_k0, xg_k12, w1g0_k12, ... so the
  first accumulation chain's operands land at the earliest possible time
  given the serial HWDGE (625ns/DMA) + DGE delay (650) + completion-sem
  (900) pipeline. Biases ride the Pool/SWDGE path which does not contend
  for HWDGE.
- PE warmup is a chain of matmuls on a memset tile sized to end right at
  first-data-ready: the PE p-state ramp (half clock for the first 3us of
  busy) burns on dummy work during the dead DMA window, and the handoff
  to real matmuls is gap-free.
- fc1 groups are emitted k-major (all 4 f-chains advance together) to
  match the k-slice DMA arrival order.
"""

import ml_dtypes
import numpy as np

import concourse.tile as tile
from concourse import bacc, mybir
from concourse.bass_utils import run_bass_kernel_spmd

N, H, F, E = 2048, 1024, 4096, 8
NCORES = 8
P = 128
KH = H // P               # 8  k-chunks for fc1 (contract over H)
KF = F // P               # 32 k-chunks for fc2 (contract over F)
FT = F // P               # 32 f-tiles of hT
HT = H // P               # 8  h-tiles of yT
FG = 8                    # fc1 f-tile groups (4 f-tiles = 512 cols each)
GW = F // FG              # 512 cols of W1 per group
GF = GW // P              # 4 f-tiles per group

TAILF = 6                 # fc2 tail chunks run per-h so output acts/DMAs
                          # stagger instead of piling up at the end
NWARM = 14                # warmup matmuls (PE p-state ramp burn), 128 cols each
W2_T0 = 0.016             # first w2 DMA release time (ms ~ us*1e-3)
W2_DT = 0.0007            # per-tile w2 release stagger

BF16 = ml_dtypes.bfloat16

_compiled = {}
_last_nc = None


def _build(C: int, repeats: int = 1):
    """Build + compile the SPMD program for token capacity C (<=512)."""
    nc = bacc.Bacc("TRN2", target_bir_lowering=False, debug=False,
                   num_devices=NCORES)
    f32 = mybir.dt.float32
    bf16 = mybir.dt.bfloat16
    GELU = mybir.ActivationFunctionType.Gelu_apprx_tanh
    IDENT = mybir.ActivationFunctionType.Identity

    xg = nc.dram_tensor("xg", [P, KH, C], bf16, kind="ExternalInput").ap()
    w1 = nc.dram_tensor("w1", [FG, P, KH, GW], bf16, kind="ExternalInput").ap()
    b1 = nc.dram_tensor("b1", [P, FT], f32, kind="ExternalInput").ap()
    w2 = nc.dram_tensor("w2", [KF, P, H], bf16, kind="ExternalInput").ap()
    b2 = nc.dram_tensor("b2", [P, HT], f32, kind="ExternalInput").ap()
    y = nc.dram_tensor("y", [HT, P, C], bf16, kind="ExternalOutput").ap()

    with tile.TileContext(nc) as tc:
      for _rep in range(repeats):
        with (
            tc.tile_pool(name="singles", bufs=1) as singles,
            tc.tile_pool(name="w1p", bufs=3) as w1p,
            tc.tile_pool(name="w2p", bufs=10) as w2p,
            tc.tile_pool(name="outp", bufs=8) as outp,
            tc.tile_pool(name="ps", bufs=8, space="PSUM") as ps,
        ):
            # PE warmup: a chain of dummy matmuls on a memset tile fills the
            # startup DMA window, burning the p-state ramp (half clock for
            # the first ~3us of PE-busy) on dummy work so real matmuls run
            # at full clock. Sized to end right at first-data-ready so the
            # warmup->real handoff has no PE idle gap.
            warm = singles.tile([P, P], bf16)
            nc.vector.memset(warm, 0.0)
            for i in range(NWARM):
                wps = ps.tile([P, P], f32, tag="ps", name=f"wps{i}")
                nc.tensor.matmul(out=wps, lhsT=warm, rhs=warm,
                                 start=True, stop=True)

            # Startup feed, all on the SP queue (serial HWDGE): alternate
            # 2-k-slice xg/w1 pieces so the k-slices both chains need
            # arrive in consumption order. 2 slices/piece balances the
            # serial HWDGE rate (625ns/DMA) against the serial transfer
            # rate (~360ns/slice).
            xg_s = singles.tile([P, KH, C], bf16)
            w1_g0 = w1p.tile([P, KH, GW], bf16, name="w1g0", tag="w1")
            for k in range(0, KH, 2):
                nc.sync.dma_start(out=xg_s[:, k:k + 2, :], in_=xg[:, k:k + 2, :])
                nc.sync.dma_start(out=w1_g0[:, k:k + 2, :],
                                  in_=w1[0, :, k:k + 2, :])
            # Biases ride the Pool/SWDGE path: no HWDGE contention.
            b1_s = singles.tile([P, FT], f32)
            b2_s = singles.tile([P, HT], f32)
            nc.gpsimd.dma_start(out=b1_s, in_=b1)
            nc.gpsimd.dma_start(out=b2_s, in_=b2)
            hT_s = singles.tile([P, FT, C], bf16)

            def fc1_group(fg, w1_t):
                if w1_t is None:
                    w1_t = w1p.tile([P, KH, GW], bf16, name=f"w1g{fg}",
                                    tag="w1")
                    # group 1 in 4 pieces: its consumption window starts
                    # right after group 0, before the transfer queue has
                    # drained -- finer completion-sems hide the 900ns lag
                    step = 2 if fg == 1 else 4
                    for k in range(0, KH, step):
                        nc.sync.dma_start(out=w1_t[:, k:k + step, :],
                                          in_=w1[fg, :, k:k + step, :])
                # k-major emission: all 4 f-chains advance together so
                # consumption matches the k-slice DMA arrival order.
                pst = [ps.tile([P, C], f32, tag="ps", name=f"ps{fg}_{fl}")
                       for fl in range(GF)]
                for k in range(KH):
                    for fl in range(GF):
                        nc.tensor.matmul(
                            out=pst[fl],
                            lhsT=w1_t[:, k, fl * P:(fl + 1) * P],
                            rhs=xg_s[:, k, :],
                            start=(k == 0), stop=(k == KH - 1))
                for fl in range(GF):
                    ft = fg * GF + fl
                    nc.scalar.activation(
                        out=hT_s[:, ft, :], in_=pst[fl], func=GELU,
                        bias=b1_s[:, ft:ft + 1])

            fc1_group(0, w1_g0)
            for fg in range(1, FG):
                fc1_group(fg, None)

            # fc2: 8 accumulators (one per h-tile) over 32 f-chunks. The
            # shared PSUM pool means each ps_y waits only for its own
            # bank's last gelu, so the first fc2 matmul issues right after
            # fc1's last -- no pool-close barrier.
            ps_y = [ps.tile([P, C], f32, tag="ps", name=f"ps_y{h}")
                    for h in range(HT)]
            w2_t = []
            for f in range(KF):
                t = w2p.tile([P, H], bf16, tag="w2", name="w2t")
                # w2 released only after the startup window so it cannot
                # steal HWDGE/DMA slots from the fc1 feed.
                with tc.tile_wait_until(W2_T0 + f * W2_DT):
                    nc.scalar.dma_start(out=t, in_=w2[f])
                w2_t.append(t)
                if f < KF - TAILF:
                    for h in range(HT):
                        nc.tensor.matmul(
                            out=ps_y[h], lhsT=t[:, h * P:(h + 1) * P],
                            rhs=hT_s[:, f, :], start=(f == 0), stop=False)
            for h in range(HT):
                for f in range(KF - TAILF, KF):
                    nc.tensor.matmul(
                        out=ps_y[h], lhsT=w2_t[f][:, h * P:(h + 1) * P],
                        rhs=hT_s[:, f, :], start=False, stop=(f == KF - 1))
                o_t = outp.tile([P, C], bf16, tag="y", name=f"o{h}")
                nc.scalar.activation(out=o_t, in_=ps_y[h], func=IDENT,
                                     bias=b2_s[:, h:h + 1])
                (nc.sync if h % 2 == 0 else nc.scalar).dma_start(
                    out=y[h], in_=o_t)

    nc.compile()
    return nc


def kernel(**inputs) -> np.ndarray:
    global _last_nc
    x = np.ascontiguousarray(np.asarray(inputs["x"], dtype=np.float32))
    mapping = np.asarray(inputs["mapping"]).astype(np.int64)
    Wg = np.asarray(inputs["Wg"], dtype=np.float32)
    W1 = np.asarray(inputs["W1"], dtype=np.float32)
    b1 = np.asarray(inputs["b1"], dtype=np.float32)
    W2 = np.asarray(inputs["W2"], dtype=np.float32)
    b2 = np.asarray(inputs["b2"], dtype=np.float32)

    n, h = x.shape
    assert (n, h) == (N, H)

    # Host-side dispatch: unique tokens per expert (a token routed to the
    # same expert by both slots contributes once, with summed gate weight).
    token_lists = []
    for e in range(E):
        tl = np.nonzero((mapping == e).any(axis=1))[0]
        token_lists.append(tl)
    maxc = max(len(tl) for tl in token_lists)
    C = max(256, -(-maxc // 8) * 8)
    assert C <= 512, f"per-expert token count {maxc} exceeds single-chunk capacity"

    if C not in _compiled:
        _compiled[C] = _build(C)
    nc = _compiled[C]
    _last_nc = nc

    in_maps = []
    for e in range(E):
        tl = token_lists[e]
        xgT = np.zeros((H, C), dtype=BF16)
        xgT[:, :len(tl)] = x[tl].T.astype(BF16)
        in_maps.append({
            # [P, KH, C]: xg[r, k, c] = x[tl[c], k*128+r]
            "xg": np.ascontiguousarray(xgT.reshape(KH, P, C).transpose(1, 0, 2)),
            # [FG, P, KH, GW]: w1[fg, r, k, c] = W1[k*128+r, fg*512+c]
            "w1": np.ascontiguousarray(
                W1[e].reshape(KH, P, FG, GW).transpose(2, 1, 0, 3)).astype(BF16),
            "b1": np.ascontiguousarray(b1[e].reshape(FT, P).T),
            "w2": W2[e].reshape(KF, P, H).astype(BF16),
            "b2": np.ascontiguousarray(b2[e].reshape(HT, P).T),
        })

    res = run_bass_kernel_spmd(nc, in_maps, list(range(NCORES)))

    # Host: gate GEMM + token-axis softmax, per-(token,k) weights, combine.
    zf = x @ Wg.T                                     # [N, E]
    zf -= zf.max(axis=0, keepdims=True)
    ez = np.exp(zf)
    logits = ez / ez.sum(axis=0, keepdims=True)
    w = np.take_along_axis(logits, mapping, axis=1)
    w = w / w.sum(axis=1, keepdims=True)

    out = np.zeros((N, H), dtype=np.float32)
    for e in range(E):
        tl = token_lists[e]
        yT = res.results[e]["y"].astype(np.float32).reshape(H, -1)
        cw = (w[tl, 0] * (mapping[tl, 0] == e)
              + w[tl, 1] * (mapping[tl, 1] == e)).astype(np.float32)
        out[tl] += cw[:, None] * yT[:, :len(tl)].T
    return out
